# revision 26
# baseline (speedup 1.0000x reference)
"""LookupConv2d kernel for Trainium2 (8 NeuronCores, data-parallel over batch).

Computation: weight[o] = sum_s coeff[o,s] * dictionary[idx[o,s]]  (tiny, host)
             out = conv2d(x, weight, stride 1, pad 1)             (device)

Device strategy per core (4 images each), ~113us vs 140us fp32r baseline:
  - x is host-padded to (58,58), cast to bf16, laid out (Cin=128, img, H+2, W+2).
  - Conv = 9 shifted matmuls accumulated in PSUM: per output tile of 8 rows x
    56 cols (N=448), rhs = padded-x window (8x56 stride-58 AP), lhsT =
    (Cin=128, Cout-half=128) tap weights.  bf16 streams 1 col/cycle like
    fp32r (cost model: both 1.0 cyc/row) but halves DMA bytes, loads weights
    faster (back-to-back spacing 190ns vs 210ns) and throttles less (DVFS
    time 6us vs 18us).  fp8 would be 2x but fails the 2e-2 gate (3.6e-2).
  - Tap chains start with the never-padded (ky=1,kx=1) tap so the other taps
    can trim rows/cols that only multiply zero padding (~2.4% fewer cols).
  - Input DMAs are split across the two HWDGE families (w+rest on SP, x0
    rows 0-17 on Activation) in consumption order, with 2-column "absorber"
    matmuls interleaved so every real matmul carries at most one sync wait
    (the ISA allows only one per instruction).  Tiny warm DMAs trigger each
    family's ring init early.
  - ~3.8us of dummy matmuls on a memset tile fill the PE-idle input window:
    the DVFS clock ramps only while PE is continuously busy (0.65 -> 1.2 ->
    2.4 GHz after 3us) and resets on any idle gap, so without them the first
    ~18 real matmuls run at 1/3 clock.
  - Outputs stream as bf16 (host upcasts): per (img, co-half, nt-pair) the
    PSUM tiles are cast by DVE into their own SBUF staging slot and DMA'd on
    the Activation FIFO (3584B lines DMA ~2x faster than 1792B); the final
    chunk is a single nt so the end-of-kernel drain is short.
  - Output DRAM layout is partition-major (p, img, co, pix); the host
    untransposes to (img, co*128+p, pix).
"""

import numpy as np
from contextlib import ExitStack

import concourse.bass as bass
import concourse.bacc as bacc
import concourse.tile as tile
from concourse import mybir
from concourse.bass_utils import run_bass_kernel_spmd

N_CORES = 8
B, CIN, H, W = 32, 128, 56, 56
COUT = 256
KK = 3  # kernel size
HP, WP = H + 2, W + 2  # padded 58, 58
BPC = B // N_CORES  # 4 images per core
RT = 8  # output rows per matmul tile
NT = H // RT  # 7 row tiles
NPIX = RT * W  # 448 = matmul free dim
MM_DT = mybir.dt.bfloat16
F32 = mybir.dt.float32

_CACHE: dict = {}


def _build_program():
    nc = bacc.Bacc("TRN2", target_bir_lowering=False, debug=False)
    xs = nc.dram_tensor("xs", [CIN, BPC, HP, WP], MM_DT, kind="ExternalInput")
    # weights co-half-major: (Cin, co_half, tap, m)
    wt = nc.dram_tensor("wt", [CIN, 2, KK * KK, 128], MM_DT, kind="ExternalInput")
    # bf16 output (host upcasts): halves the output DMA bytes and doubles
    # DVE copy throughput; adds ~1e-3 rel err on top of the bf16-input 2.3e-3.
    out = nc.dram_tensor("out", [CIN, BPC * 2 * H * W], MM_DT, kind="ExternalOutput")

    with tile.TileContext(nc) as tc, ExitStack() as ctx:
        xpool = ctx.enter_context(tc.tile_pool(name="x", bufs=1))
        wpool = ctx.enter_context(tc.tile_pool(name="w", bufs=1))
        dpool = ctx.enter_context(tc.tile_pool(name="d", bufs=1))
        # One staging tile per output chunk: no slot recycling, so each DVE
        # copy owes exactly one sync wait (its matmul group).
        opool = ctx.enter_context(tc.tile_pool(name="o", bufs=BPC * 2 * 4))
        ppool = ctx.enter_context(tc.tile_pool(name="p", bufs=7, space="PSUM"))
        spool = ctx.enter_context(tc.tile_pool(name="s", bufs=1, space="PSUM"))

        # A matmul may carry at most ONE sync wait.  These 2-column "absorber"
        # matmuls advance PE's clock past each input DMA so the real matmuls
        # never owe semaphores to more than one producer.
        scr = spool.tile([128, 512], F32)

        def absorb(lhs_ap, rhs_ap):
            nc.tensor.matmul(scr[:, 0:2], lhs_ap, rhs_ap, start=True, stop=True)

        # Dummy operand for PE pre-ramp matmuls (DVFS ramps only while PE is
        # continuously busy and resets on any idle gap; these fill the
        # input-DMA window 7-11us so the real stream starts at full clock).
        dummy = dpool.tile([128, 512], MM_DT)
        nc.gpsimd.memset(dummy[:, 0:128], 0.0)
        nc.gpsimd.memset(dummy[:, 128:512], 0.0)

        w_all = wpool.tile([CIN, 2, KK * KK, 128], MM_DT)
        xt = xpool.tile([CIN, BPC, HP, WP], MM_DT)
        warm = dpool.tile([128, 4], MM_DT)

        # First two DMAs go on DIFFERENT HWDGE families (SP and Activation)
        # so their transfers overlap; the rest follow on the SP FIFO in
        # consumption order.  Outputs stream on the Activation FIFO, whose
        # first out-chunk comes long after x0a's transfer finished.
        # The tiny "warm" DMAs trigger each family's ring initialization
        # (~2us on first use) before the real transfers queue up behind it.
        nc.scalar.dma_start(warm[:, 0:2], wt[:, 0, 0, 0:2])
        nc.sync.dma_start(warm[:, 2:4], wt[:, 0, 0, 2:4])
        nc.sync.dma_start(w_all[:, 0:1], wt[:, 0:1])
        nc.scalar.dma_start(xt[:, 0:1, 0:18], xs[:, 0:1, 0:18])
        nc.sync.dma_start(xt[:, 0:1, 18:HP], xs[:, 0:1, 18:HP])
        nc.sync.dma_start(w_all[:, 1:2], wt[:, 1:2])
        nc.sync.dma_start(xt[:, 1:2], xs[:, 1:2])
        nc.sync.dma_start(xt[:, 2:3], xs[:, 2:3])
        nc.sync.dma_start(xt[:, 3:4], xs[:, 3:4])

        # Pre-ramp chain: ~3.8us of dummy matmuls tuned to end just past the
        # first input's arrival (~10.8us); undershooting would idle PE and
        # drop the clock back to the low p-state for the first real matmuls.
        nc.tensor.matmul(scr[:, 0:128], dummy[:, 0:128], dummy[:, 0:128],
                         start=True, stop=True)
        for _ in range(3):
            nc.tensor.matmul(scr[:], dummy[:, 0:128], dummy[:], start=True, stop=True)
        for _ in range(13):
            nc.tensor.matmul(scr[:, 0:256], dummy[:, 0:128], dummy[:, 0:256],
                             start=True, stop=True)
        # final pre-ramp matmul doubles as the w(co0) absorber: its only dep
        # is the w0 DMA, so it stalls exactly when absorb_w(0) would have.
        nc.tensor.matmul(scr[:, 0:256], w_all[:, 0, 0, :], dummy[:, 0:256],
                         start=True, stop=True)

        def absorb_w(co):
            absorb(w_all[:, co, 0, :], w_all[:, co, 0, 0:2])

        def absorb_x(img, r0, r1):
            # stationary = co0 weights (already observed); moving = 2 cols
            # inside the freshly DMA'd x rows [r0,r1) — the only NEW dep.
            absorb(w_all[:, 0, 0, :], xt[:, img, r0 : r0 + 1, 0:2])

        # no absorber for x0 rows 0-17: the first real matmul's only other
        # dep is the (already absorbed) w, so it carries the x0a wait itself.

        def out_base(img, co, nt):
            return img * 2 * H * W + co * H * W + nt * NPIX

        for img in range(BPC):
            for co in range(2):
                ot = None
                for nt in range(NT):
                    # interleave absorbers right before the first matmul that
                    # needs the corresponding DMA's data
                    if img == 0 and co == 1 and nt == 0:
                        absorb_w(1)
                    if img > 0 and co == 0 and nt == 0:
                        absorb_x(img, 0, HP)
                    pt = ppool.tile([128, RT, W], F32, tag="pt")
                    # Tap order starts with (ky=1,kx=1), the only tap with no
                    # zero-padding overlap, so start=True covers all 448
                    # columns; the other taps trim rows/cols that would only
                    # multiply padding (saves ~2.4% of PE columns).
                    taps = [(1, 1), (1, 0), (1, 2), (0, 0), (0, 1), (0, 2),
                            (2, 0), (2, 1), (2, 2)]
                    for i, (ky, kx) in enumerate(taps):
                        a = 1 if (nt == 0 and ky == 0) else 0
                        b = 1 if (nt == NT - 1 and ky == KK - 1) else 0
                        cl = 1 if kx == 0 else 0
                        cr = 1 if kx == KK - 1 else 0
                        r0 = nt * RT + ky + a
                        nc.tensor.matmul(
                            pt[:, a : RT - b, cl : W - cr],
                            w_all[:, co, ky * KK + kx, :],
                            xt[:, img, r0 : r0 + RT - a - b,
                               kx + cl : kx + W - cr],
                            start=(i == 0),
                            stop=(i == KK * KK - 1),
                        )
                    # Stage into nt-pair SBUF chunks (3584B per-partition
                    # lines DMA ~2x faster than 1792B) and stream out on the
                    # Activation FIFO; the final chunk is a single nt so the
                    # end-of-kernel drain is short.
                    if nt == NT - 1:
                        ot = opool.tile([128, NPIX], MM_DT, tag="ot")
                        nc.vector.tensor_copy(ot[:], pt[:])
                        base = out_base(img, co, nt)
                        nc.scalar.dma_start(out[:, base : base + NPIX], ot[:])
                    elif nt % 2 == 0:
                        ot = opool.tile([128, 2 * NPIX], MM_DT, tag="ot")
                        nc.vector.tensor_copy(ot[:, 0:NPIX], pt[:])
                    else:
                        nc.vector.tensor_copy(ot[:, NPIX : 2 * NPIX], pt[:])
                        base = out_base(img, co, nt - 1)
                        nc.scalar.dma_start(
                            out[:, base : base + 2 * NPIX], ot[:]
                        )
    nc.compile()
    return nc


def _get_program():
    if "nc" not in _CACHE:
        _CACHE["nc"] = _build_program()
    return _CACHE["nc"]


def _prepare_inputs(x, dictionary, lookup_coefficients, lookup_indices):
    import ml_dtypes

    x = np.asarray(x, dtype=np.float32)
    dictionary = np.asarray(dictionary, dtype=np.float32)
    coeff = np.asarray(lookup_coefficients, dtype=np.float32)
    idx = np.asarray(lookup_indices)

    # Compose per-output-channel filters on host (2.4 MFLOP — negligible).
    atoms = dictionary[idx]  # (Cout, S, Cin, K, K)
    weight = np.einsum("os,osckl->ockl", coeff, atoms)  # (Cout, Cin, K, K)
    # lhsT layout co-half-major: (Cin, co_half, tap, m)
    wt_host = np.ascontiguousarray(
        weight.reshape(2, 128, CIN, KK * KK).transpose(2, 0, 3, 1)
    ).astype(ml_dtypes.bfloat16)

    # Zero-pad spatially, then per-core layout (Cin, img, HP, WP) in bf16.
    x_pad = np.zeros((B, CIN, HP, WP), dtype=np.float32)
    x_pad[:, :, 1 : H + 1, 1 : W + 1] = x
    x_pad = x_pad.astype(ml_dtypes.bfloat16)
    in_maps = []
    for c in range(N_CORES):
        xs_core = np.ascontiguousarray(
            x_pad[c * BPC : (c + 1) * BPC].transpose(1, 0, 2, 3)
        )
        in_maps.append({"xs": xs_core, "wt": wt_host})
    return in_maps


def _ensure_ntff_hook() -> bool:
    """Register the axon NTFF profile hook (missing antenv.axon_hooks shim).

    Only needed for trace=True runs; grading path (trace=False) never calls it.
    """
    import sys
    import types
    import contextlib
    import ctypes

    try:
        import antenv.axon_hooks as m  # noqa: F401
        if m.get_axon_ntff_profile_hook() is not None:
            return True
    except ImportError:
        m = types.ModuleType("antenv.axon_hooks")
        _h = {"hook": None}
        m.set_axon_ntff_profile_hook = lambda h: _h.__setitem__("hook", h)
        m.get_axon_ntff_profile_hook = lambda: _h["hook"]
        sys.modules["antenv.axon_hooks"] = m
        try:
            import antenv
            antenv.axon_hooks = m
        except ImportError:
            pass

    so_path = "/opt/axon/libaxon_pjrt.so"
    try:
        lib = ctypes.CDLL(so_path)
    except OSError:
        return False
    if not hasattr(lib, "axon_start_nrt_profile"):
        return False
    lib.axon_start_nrt_profile.argtypes = [
        ctypes.POINTER(ctypes.c_int64),
        ctypes.c_size_t,
    ]
    lib.axon_start_nrt_profile.restype = ctypes.c_int64
    lib.axon_stop_nrt_profile.argtypes = [ctypes.c_char_p]
    lib.axon_stop_nrt_profile.restype = ctypes.c_int64

    @contextlib.contextmanager
    def _hook(output_dir, device_ids):
        import jax

        jax.devices()
        if device_ids:
            ids = (ctypes.c_int64 * len(device_ids))(*device_ids)
            rc = lib.axon_start_nrt_profile(ids, len(device_ids))
        else:
            rc = lib.axon_start_nrt_profile(None, 0)
        if rc != 0:
            raise RuntimeError(f"axon_start_nrt_profile rc={rc}")
        try:
            yield
        finally:
            n = lib.axon_stop_nrt_profile(str(output_dir).encode())
            if n < 0:
                raise RuntimeError(f"axon_stop_nrt_profile rc={n}")
            print(f"profile: {n} file(s) written to {output_dir}", file=sys.stderr)

    m.set_axon_ntff_profile_hook(_hook)
    return True


def _run(inputs: dict, trace: bool = False):
    if trace:
        trace = _ensure_ntff_hook()
    nc = _get_program()
    in_maps = _prepare_inputs(**inputs)
    res = run_bass_kernel_spmd(nc, in_maps, list(range(N_CORES)), trace=trace)
    out = np.empty((B, COUT, H, W), dtype=np.float32)
    for c in range(N_CORES):
        # device layout: (p=128, img, co, pix) -> (img, co*128+p, h, w)
        arr = res.results[c]["out"].astype(np.float32).reshape(128, BPC, 2, H * W)
        out[c * BPC : (c + 1) * BPC] = (
            arr.transpose(1, 2, 0, 3).reshape(BPC, COUT, H, W)
        )
    return out, res


def kernel(**inputs) -> np.ndarray:
    out, _ = _run(inputs, trace=False)
    return out
